# revision 1
# baseline (speedup 1.0000x reference)
"""ConvPMF forward on 8 Trainium2 NeuronCores (Bass/Tile).

Per core (data-parallel over the valid (batch, review) pairs):
  1. indirect-DMA gather of word embeddings, one [128 words, 128 dim] tile
     per instruction (the SWDGE per-instruction cost is the kernel's
     critical path; everything else hides under the gather stream)
  2. PE transpose -> rev [dim, words] (fp32)
  3. Conv1d(SAME) as 5 PSUM-accumulated fp32 matmuls with shifted/clipped
     rhs windows; 4 reviews run concurrently on the tensor engine via
     column tiling (M=32 each). The dense fp32 matmul stream also keeps the
     PE HAM clock-gate warm, which minimizes the post-gather tail.
  4. max-softmax pool:  max(softmax(fm)) == 1 / sum_w exp(fm - max_w fm)
     -> reduce_max (DVE), Exp with accum_out (ACT), reciprocal (DVE)
Host: shard the ragged review list, combine pooled vectors into item
embeddings, dot with user factors, add bias.
"""
import math

import numpy as np

import concourse.bass as bass
import concourse.mybir as mybir
import concourse.tile as tile
from concourse import bacc
from concourse.bass import IndirectOffsetOnAxis
from concourse.bass_utils import run_bass_kernel_spmd

f32 = mybir.dt.float32
i32 = mybir.dt.int32

N_CORES = 8
D, F, K = 128, 32, 5           # embed dim, factors (conv out channels), window
W = 256                        # words per review
VOCAB = 50000
TAP_ORDER = (2, 0, 1, 3, 4)    # tap 2 covers full width -> start=True first

_program_cache: dict[int, bass.Bass] = {}


def _build_program(groups: int) -> bass.Bass:
    """One SPMD program, identical on all cores: `groups` groups of 4
    reviews, review slot j of a group on tensor-engine column group j."""
    wt = 8 * groups            # 128-word gather tiles per core

    nc = bacc.Bacc("TRN2", target_bir_lowering=False, debug=False)
    embed_d = nc.dram_tensor("embed", [VOCAB, D], f32, kind="ExternalInput")
    idx_d = nc.dram_tensor("idx", [128, wt], i32, kind="ExternalInput")
    # identity (transpose helper) ++ 5 taps of W_k^T [128, 32]
    cst_d = nc.dram_tensor("cst", [128, 128 + K * F], f32, kind="ExternalInput")
    pooled_d = nc.dram_tensor("pooled", [128, groups], f32, kind="ExternalOutput")

    with tile.TileContext(nc) as tc:
        with tc.tile_pool(name="const", bufs=1) as cpool, \
             tc.tile_pool(name="gat", bufs=4) as gpool, \
             tc.tile_pool(name="rev", bufs=2) as rpool, \
             tc.tile_pool(name="wrk", bufs=2) as wpool, \
             tc.tile_pool(name="psT", bufs=3, space="PSUM") as tppool, \
             tc.tile_pool(name="psF", bufs=2, space="PSUM") as fmpool:
            idx_sb = cpool.tile([128, wt], i32)
            c0 = min(8, wt)      # small first chunk so gather 0 starts ASAP
            nc.sync.dma_start(idx_sb[:, 0:c0], idx_d[:, 0:c0])
            if wt > c0:
                nc.sync.dma_start(idx_sb[:, c0:wt], idx_d[:, c0:wt])
            cst_sb = cpool.tile([128, 128 + K * F], f32)
            nc.sync.dma_start(cst_sb[:], cst_d[:])
            ident = cst_sb[:, 0:128]
            pooled_sb = cpool.tile([128, groups], f32)

            def wk(k):
                return cst_sb[:, 128 + k * F:128 + (k + 1) * F]

            for g in range(groups):
                g_sb = gpool.tile([128, 8 * D], f32, tag="g")
                for t in range(8):
                    nc.gpsimd.indirect_dma_start(
                        out=g_sb[:, t * D:(t + 1) * D], out_offset=None,
                        in_=embed_d[:],
                        in_offset=IndirectOffsetOnAxis(
                            ap=idx_sb[:, 8 * g + t:8 * g + t + 1], axis=0))

                revs = []
                for j in range(4):
                    rev_ps = tppool.tile([128, W], f32, tag="revps")
                    nc.tensor.transpose(
                        rev_ps[:, 0:128], g_sb[:, (2 * j) * D:(2 * j + 1) * D],
                        ident)
                    nc.tensor.transpose(
                        rev_ps[:, 128:256],
                        g_sb[:, (2 * j + 1) * D:(2 * j + 2) * D], ident)
                    rev_sb = rpool.tile([128, W], f32, tag=f"rev{j}")
                    if j % 2 == 0:
                        nc.vector.tensor_copy(rev_sb[:], rev_ps[:])
                    else:
                        nc.scalar.copy(rev_sb[:], rev_ps[:])
                    revs.append(rev_sb)

                fm_ps = fmpool.tile([128, W], f32, tag="fm")
                for k in TAP_ORDER:
                    s = k - 2                     # word shift of this tap
                    a, ob = max(0, s), max(0, -s)
                    n = W - abs(s)
                    for j in range(4):
                        nc.tensor.matmul(
                            fm_ps[32 * j:32 * j + 32, ob:ob + n],
                            lhsT=wk(k), rhs=revs[j][:, a:a + n],
                            start=(k == TAP_ORDER[0]), stop=(k == TAP_ORDER[-1]),
                            tile_position=(0, 32 * j))

                negm = wpool.tile([128, 1], f32, tag="negm")
                nc.vector.tensor_reduce(
                    negm[:], fm_ps[:], axis=mybir.AxisListType.X,
                    op=mybir.AluOpType.max, negate=True)
                e_sb = wpool.tile([128, W], f32, tag="e")
                ssum = wpool.tile([128, 1], f32, tag="ssum")
                nc.scalar.activation(
                    e_sb[:], fm_ps[:], mybir.ActivationFunctionType.Exp,
                    bias=negm[:], scale=1.0, accum_out=ssum[:])
                nc.vector.reciprocal(pooled_sb[:, g:g + 1], ssum[:])
                nc.sync.dma_start(pooled_d[:, g:g + 1], pooled_sb[:, g:g + 1])
    nc.compile()
    return nc


def prepare(user_indices, docs, review_counts, w_user, embed_matrix,
            conv_weight, bias):
    """Host-side sharding prep: returns (nc, in_maps, valid, n_core) or None
    when there are no valid reviews."""
    docs = np.asarray(docs)
    review_counts = np.asarray(review_counts)
    embed_matrix = np.ascontiguousarray(np.asarray(embed_matrix, dtype=np.float32))
    conv_weight = np.asarray(conv_weight, dtype=np.float32)

    b_sz = docs.shape[0]
    valid = [(b, r) for b in range(b_sz) for r in range(int(review_counts[b]))]
    if not valid:
        return None

    n_core = math.ceil(len(valid) / N_CORES)
    groups = math.ceil(n_core / 4)
    n_core = groups * 4

    cst = np.zeros((128, 128 + K * F), dtype=np.float32)
    cst[:, 0:128] = np.eye(128, dtype=np.float32)
    for k in range(K):
        cst[:, 128 + k * F:128 + (k + 1) * F] = conv_weight[:, :, k].T

    docs32 = docs.astype(np.int32)
    in_maps = []
    for c in range(N_CORES):
        idx = np.zeros((128, 8 * groups), dtype=np.int32)
        for slot in range(n_core):
            i = c * n_core + slot
            if i >= len(valid):
                break
            bb, rr = valid[i]
            wrds = docs32[bb, rr]                      # [256]
            g, j = slot // 4, slot % 4
            idx[:, 8 * g + 2 * j] = wrds[0:128]
            idx[:, 8 * g + 2 * j + 1] = wrds[128:256]
        in_maps.append({"embed": embed_matrix, "idx": idx, "cst": cst})

    nc = _program_cache.get(groups)
    if nc is None:
        nc = _build_program(groups)
        _program_cache[groups] = nc
    return nc, in_maps, valid, n_core


def kernel(user_indices, docs, review_counts, w_user, embed_matrix, conv_weight,
           bias):
    user_indices = np.asarray(user_indices)
    docs = np.asarray(docs)
    review_counts = np.asarray(review_counts)
    w_user = np.asarray(w_user, dtype=np.float32)
    conv_weight = np.asarray(conv_weight, dtype=np.float32)
    bias = np.asarray(bias, dtype=np.float32)

    b_sz = docs.shape[0]
    denom = np.maximum(review_counts, 1).astype(np.float32)
    prep = prepare(user_indices, docs, review_counts, w_user, embed_matrix,
                   conv_weight, bias)
    if prep is None:
        return np.full((b_sz,), bias[0], dtype=np.float32)
    nc, in_maps, valid, n_core = prep

    res = run_bass_kernel_spmd(nc, in_maps, list(range(N_CORES)))

    item = np.zeros((b_sz, F), dtype=np.float32)
    for i, (bb, rr) in enumerate(valid):
        c, slot = i // n_core, i % n_core
        g, j = slot // 4, slot % 4
        item[bb] += res.results[c]["pooled"][32 * j:32 * j + 32, g]
    item /= denom[:, None]
    out = (w_user[user_indices] * item).sum(axis=-1) + bias[0]
    return out.astype(np.float32)

